# revision 16
# baseline (speedup 1.0000x reference)
"""Multi-head attention (B=4, S=2048, D=2048, H=16) on 8 TRN2 NeuronCores.

Sharding: core c handles batch b=c//2 and head-group g=c%2 (8 of 16 heads).
Each core computes Q/K/V projections for its head group, attention, and a
partial O-projection (its heads' contribution to the full (S, D) output).
Host sums the two partials per batch and adds the output bias.

v2 design (vs the fp32r baseline):
 - ALL matmuls in bf16: same 1 cyc/row streaming as fp32r, but LDWEIGHTS
   gets Fast Weight Load (~4x cheaper than fp32r's ~200ns — the trace
   showed 265us of serialized fp32r LDWEIGHTS), and DMA/SBUF halve.
 - Everything stays resident in SBUF: x (64K/part), v (32K), q/k (rolling),
   o-partials (32K) — no DRAM round-trips for intermediates at all.
 - x is consumed *while it loads*: head 0's Q/K projections accumulate in
   PSUM as each x row-tile lands (x-load DMA ~25us == head-0 QK PE time).
 - Q/K projections of head h+1 are interleaved into the attention inner
   loop of head h as PE filler, so the scalar engine's exp latency
   (~1us per 128x1024 tile, the co-bottleneck of the baseline's attention
   phase) hides entirely under tensor work.
 - Softmax denominator: DVE accumulates exp tiles in bf16 (2x mode),
   one ones-matmul per query block partition-reduces it, DVE reciprocal
   (the recommended path) replaces the baseline's ln/exp chain, and the
   normalize multiply runs bf16 SBUF->SBUF at 2x off the critical path
   (PV output is staged raw via a scalar-engine copy to free PSUM fast).

PSUM (8 banks): tag S = [128,1024] x2 (scores / denom / V-proj / O-proj),
tag O = x1 (PV accumulator / warmup), tag Q = x1 (interleaved QK halves).
"""
import sys

for _p in ("/opt/trn_rl_repo", "/root/.axon_site/_ro/trn_rl_repo"):
    if _p not in sys.path:
        sys.path.insert(0, _p)

import numpy as np
import concourse.bass as bass
import concourse.tile as tile
import concourse.mybir as mybir
from concourse.bass_utils import run_bass_kernel_spmd

F32 = mybir.dt.float32
BF16 = mybir.dt.bfloat16
NPBF16 = mybir.dt.np(BF16)
AFT = mybir.ActivationFunctionType
ADD = mybir.AluOpType.add
MUL = mybir.AluOpType.mult

D = 2048       # model dim
S = 2048       # sequence length
B = 4          # batch
HPC = 8        # heads per core (16 heads / 2 tensor-parallel groups)
DH = 128       # head dim
GW = HPC * DH  # head-group width = 1024
N_CORES = 8
P = 128        # partitions
QB = 512       # matmul free-dim block (one PSUM bank of fp32)
NKT = S // P   # 16 key tiles
NKC = D // P   # 16 contraction chunks over model dim


def split_waits(nc, max_waits=1):
    """This walrus build rejects instructions with >1 semaphore wait
    (setupSyncWait: 'Too many sync wait commands').  Move extra waits onto
    NoOp instructions inserted just before, on the same engine stream —
    semantically identical (the engine stalls on the NoOps first)."""
    n = 0
    for fn in nc.m.functions:
        for blk in fn.blocks:
            out = []
            for inst in blk.instructions:
                si = inst.sync_info
                waits = list(si.on_wait) if si and si.on_wait else []
                eng = inst.engine
                if len(waits) > max_waits and eng != mybir.EngineType.Unassigned:
                    for w in waits[:-max_waits]:
                        nop = mybir.InstNoOp(name=nc.get_next_instruction_name())
                        nop.engine = eng
                        nop.sync_info = mybir.SyncInfo(on_wait=[w], on_update=[])
                        out.append(nop)
                        n += 1
                    inst.sync_info = mybir.SyncInfo(
                        on_wait=waits[-max_waits:],
                        on_update=list(si.on_update) if si.on_update else [],
                    )
                out.append(inst)
            blk.instructions = out
    return n


def build(masked=False, split=True):
    nc = bass.Bass("TRN2", target_bir_lowering=False, debug=False,
                   num_devices=N_CORES)

    xT = nc.dram_tensor("xT", [D, S], BF16, kind="ExternalInput").ap()
    WqP = nc.dram_tensor("WqP", [HPC, P, NKC, P], BF16, kind="ExternalInput").ap()
    WkP = nc.dram_tensor("WkP", [HPC, P, NKC, P], BF16, kind="ExternalInput").ap()
    WvP = nc.dram_tensor("WvP", [8, P, 2, GW], BF16, kind="ExternalInput").ap()
    WoT = nc.dram_tensor("WoT", [GW, D], BF16, kind="ExternalInput").ap()
    bqs = nc.dram_tensor("bqs", [P, HPC], F32, kind="ExternalInput").ap()
    bks = nc.dram_tensor("bks", [P, HPC], F32, kind="ExternalInput").ap()
    bv = nc.dram_tensor("bv", [P, GW], BF16, kind="ExternalInput").ap()
    ones = nc.dram_tensor("ones", [P, P], BF16, kind="ExternalInput").ap()
    if masked:
        maskT = nc.dram_tensor("maskT", [S, S], BF16, kind="ExternalInput").ap()
    out = nc.dram_tensor("out", [S, D], BF16, kind="ExternalOutput").ap()

    with tile.TileContext(nc) as tc:
        with (
            tc.tile_pool(name="xpool", bufs=16) as xpool,
            tc.tile_pool(name="vpool", bufs=1) as vpool,
            tc.tile_pool(name="qkp", bufs=2) as qkp,
            tc.tile_pool(name="wqkp", bufs=1) as wqkp,
            tc.tile_pool(name="wvp", bufs=8) as wvp,
            tc.tile_pool(name="opool", bufs=16) as opool,
            tc.tile_pool(name="orawp", bufs=1) as orawp,
            tc.tile_pool(name="expp", bufs=2) as expp,
            tc.tile_pool(name="accp", bufs=1) as accp,
            tc.tile_pool(name="recp", bufs=2) as recp,
            tc.tile_pool(name="stagep", bufs=2) as stagep,
            tc.tile_pool(name="misc", bufs=1) as misc,
            tc.tile_pool(name="psum", bufs=1, space="PSUM") as psum,
        ):
            def psS():
                return psum.tile([P, 2 * QB], F32, tag="S", bufs=2, name="psS")

            def psO():
                return psum.tile([P, 2 * QB], F32, tag="O", bufs=1, name="psO")

            def psQ():
                return psum.tile([P, 2 * QB], F32, tag="Q", bufs=1, name="psQ")

            # ---- constants / biases --------------------------------------
            ones_t = misc.tile([P, P], BF16)
            nc.gpsimd.dma_start(ones_t[:], ones[:])
            bqs_t = misc.tile([P, HPC], F32)
            nc.gpsimd.dma_start(bqs_t[:], bqs[:])
            bks_t = misc.tile([P, HPC], F32)
            nc.gpsimd.dma_start(bks_t[:], bks[:])

            # ---- x load + head-0 Q/K projections (consume x as it lands) -
            # weight DMAs first (small, needed by the first matmul), then
            # the x row-tiles; Wv starts mid-x-load so it lands by V-phase.
            wq0 = wqkp.tile([P, NKC, P], BF16, tag="wq", name="wq0")
            nc.gpsimd.dma_start(wq0[:], WqP[0])
            wk0 = wqkp.tile([P, NKC, P], BF16, tag="wk", name="wk0")
            nc.gpsimd.dma_start(wk0[:], WkP[0])
            xk = []
            wv = []
            for ki in range(NKC):
                t = xpool.tile([P, S], BF16, tag="x", name=f"xk{ki}")
                nc.gpsimd.dma_start(t[:], xT[ki * P:(ki + 1) * P, :])
                xk.append(t)
                if ki >= 10 and len(wv) < 8:
                    w = wvp.tile([P, 2, GW], BF16, tag="wv",
                                 name=f"wv{len(wv)}")
                    nc.gpsimd.dma_start(w[:], WvP[len(wv)])
                    wv.append(w)
            while len(wv) < 8:
                w = wvp.tile([P, 2, GW], BF16, tag="wv", name=f"wv{len(wv)}")
                nc.gpsimd.dma_start(w[:], WvP[len(wv)])
                wv.append(w)
            bv_t = misc.tile([P, GW], BF16)
            nc.gpsimd.dma_start(bv_t[:], bv[:])

            # PE warm-up during the first DMA waits (HAM clock gate releases
            # to 2.4 GHz only after ~3.4us of sustained activity).
            pw = psO()
            for _ in range(24):
                nc.tensor.matmul(pw[:, :P], ones_t[:], ones_t[:],
                                 start=True, stop=True)

            psq = [psS(), psS()]
            psk = [psO(), psQ()]
            for ki in range(NKC):
                st_, sp_ = (ki == 0), (ki == NKC - 1)
                for w, pss in ((wq0, psq), (wk0, psk)):
                    lhs = w[:, ki, :]
                    for half in range(2):
                        for j in range(2):
                            c0 = half * GW + j * QB
                            nc.tensor.matmul(
                                pss[half][:, j * QB:(j + 1) * QB], lhs,
                                xk[ki][:, c0:c0 + QB], start=st_, stop=sp_)
            qh, kh = {}, {}
            qh[0] = qkp.tile([P, S], BF16, tag="q", name="qh0")
            kh[0] = qkp.tile([P, S], BF16, tag="k", name="kh0")
            for half in range(2):
                nc.scalar.activation(qh[0][:, half * GW:(half + 1) * GW],
                                     psq[half][:], AFT.Identity,
                                     bias=bqs_t[:, 0:1])
                nc.scalar.activation(kh[0][:, half * GW:(half + 1) * GW],
                                     psk[half][:], AFT.Identity,
                                     bias=bks_t[:, 0:1])

            # ---- V projection (v stays resident, natural (seq, dh)) ------
            v_all = vpool.tile([P, NKT, GW], BF16, tag="v", name="v_all")
            for st in range(NKT):
                ps = psS()
                for ki in range(NKC):
                    lhs = xk[ki][:, st * P:(st + 1) * P]
                    for j in range(2):
                        nc.tensor.matmul(
                            ps[:, j * QB:(j + 1) * QB], lhs,
                            wv[ki // 2][:, ki % 2, j * QB:(j + 1) * QB],
                            start=(ki == 0), stop=(ki == NKC - 1))
                with nc.allow_low_precision(reason="bf16 v tile"):
                    nc.vector.tensor_tensor(v_all[:, st, :], ps[:], bv_t[:],
                                            ADD)
            # Wo reuses the wv slots (same tag/size); WAR deps on the last
            # V-phase reads sequence the DMAs automatically.
            wo = []
            for h in range(HPC):
                t = wvp.tile([P, D], BF16, tag="wv", name=f"wo{h}")
                nc.gpsimd.dma_start(t[:], WoT[h * P:(h + 1) * P, :])
                wo.append(t)

            # ---- merged phase: attention(h) + QK-projections(h+1) --------
            o_t = {}
            pending_tail = []
            qk_ps = [None]  # current QK psum accumulator (tag Q, bufs=1)

            def qk_units_for(hn):
                """PE-filler units for head hn's q/k projections: 4
                subphases of 16 ki-steps each, 2 matmuls per step.  The
                weight DMAs issue eagerly (now); only the matmuls are
                deferred into the returned closures."""
                wqn = wqkp.tile([P, NKC, P], BF16, tag="wq", name=f"wq{hn}")
                nc.gpsimd.dma_start(wqn[:], WqP[hn])
                wkn = wqkp.tile([P, NKC, P], BF16, tag="wk", name=f"wk{hn}")
                nc.gpsimd.dma_start(wkn[:], WkP[hn])
                qh[hn] = qkp.tile([P, S], BF16, tag="q", name=f"qh{hn}")
                kh[hn] = qkp.tile([P, S], BF16, tag="k", name=f"kh{hn}")
                units = []
                for w, bias_t, dst in ((wqn, bqs_t, qh[hn]),
                                       (wkn, bks_t, kh[hn])):
                    for half in range(2):
                        for ki in range(NKC):
                            def unit(w=w, bias_t=bias_t, dst=dst, half=half,
                                     ki=ki, hn=hn):
                                if ki == 0:
                                    # allocated at emission time so the
                                    # tag-Q slot's WAR dep lands in order
                                    qk_ps[0] = psQ()
                                ps_qk = qk_ps[0]
                                for j in range(2):
                                    c0 = half * GW + j * QB
                                    nc.tensor.matmul(
                                        ps_qk[:, j * QB:(j + 1) * QB],
                                        w[:, ki, :], xk[ki][:, c0:c0 + QB],
                                        start=(ki == 0), stop=(ki == NKC - 1))
                                if ki == NKC - 1:
                                    nc.scalar.activation(
                                        dst[:, half * GW:(half + 1) * GW],
                                        ps_qk[:], AFT.Identity,
                                        bias=bias_t[:, hn:hn + 1])
                            units.append(unit)
                return units

            def oproj_half_unit(st, dbp):
                """One half (1024 out cols) of seq-tile st's O-projection.
                Interleaved into the last attention block, whose kt loop is
                otherwise ACT-gated (no QK filler left): tag-Q PSUM is free
                there, and the PSUM->SBUF copy goes on the idle DVE."""
                def unit():
                    qb0, stq = st // 8, st % 8
                    ps = psQ()
                    for hh in range(HPC):
                        lhs = o_t[(hh, qb0)][:, stq * P:(stq + 1) * P]
                        for j in range(2):
                            c0 = dbp * GW + j * QB
                            nc.tensor.matmul(ps[:, j * QB:(j + 1) * QB],
                                             lhs, wo[hh][:, c0:c0 + QB],
                                             start=(hh == 0),
                                             stop=(hh == HPC - 1))
                    ob = stagep.tile([P, GW], BF16, tag="s", bufs=2,
                                     name="obh")
                    with nc.allow_low_precision(reason="bf16 out tile"):
                        nc.vector.tensor_copy(ob[:], ps[:])
                    nc.gpsimd.dma_start(
                        out[st * P:(st + 1) * P, dbp * GW:(dbp + 1) * GW],
                        ob[:])
                return unit

            oq = [oproj_half_unit(st, dbp)
                  for st in range(8) for dbp in range(2)]

            for h in range(HPC):
                filler = iter(qk_units_for(h + 1) if h < HPC - 1 else [])
                for qbp in range(2):
                    acc = accp.tile([P, 2 * QB], BF16, tag="a", bufs=1,
                                    name="acc")
                    ps_o = psO()
                    for kt in range(NKT):
                        st_, sp_ = (kt == 0), (kt == NKT - 1)
                        ps_s = psS()
                        ks = kh[h][:, kt * P:(kt + 1) * P]
                        for j in range(2):
                            c0 = qbp * GW + j * QB
                            nc.tensor.matmul(ps_s[:, j * QB:(j + 1) * QB],
                                             ks, qh[h][:, c0:c0 + QB],
                                             start=True, stop=True)
                        e = expp.tile([P, 2 * QB], BF16, tag="e", bufs=2,
                                      name="e")
                        nc.scalar.activation(e[:], ps_s[:], AFT.Exp)
                        if masked:
                            m = orawp.tile([P, 2 * QB], BF16, tag="oraw",
                                           bufs=2, name="mtile")
                            nc.gpsimd.dma_start(
                                m[:], maskT[kt * P:(kt + 1) * P,
                                            qbp * GW:(qbp + 1) * GW])
                            with nc.allow_low_precision(reason="mask mul"):
                                nc.vector.tensor_tensor(e[:], e[:], m[:], MUL)
                        # deferred tail of the previous query block, spread
                        # over this block's first kts.  t_copy AND t_denom
                        # must both land at kt==0: they read the previous
                        # block's ps_o/acc, whose single-buffered slots this
                        # block's PV(kt0)/acc-copy (emitted below) reuse.
                        if pending_tail:
                            if kt == 0:
                                pending_tail.pop(0)()
                                pending_tail.pop(0)()
                            elif kt <= 2:
                                pending_tail.pop(0)()
                        # PE filler while ACT runs exp(kt): QK ki-steps for
                        # head h+1, or (last block) O-projection halves
                        if h < HPC - 1:
                            for _ in range(2):
                                u = next(filler, None)
                                if u is not None:
                                    u()
                        elif qbp == 1 and kt >= 4 and oq:
                            oq.pop(0)()
                        vs = v_all[:, kt, h * DH:(h + 1) * DH]
                        for j in range(2):
                            nc.tensor.matmul(ps_o[:, j * QB:(j + 1) * QB],
                                             vs, e[:, j * QB:(j + 1) * QB],
                                             start=st_, stop=sp_)
                        with nc.allow_low_precision(reason="bf16 denom acc"):
                            if kt == 0:
                                nc.vector.tensor_copy(acc[:], e[:])
                            else:
                                nc.vector.tensor_tensor(acc[:], acc[:], e[:],
                                                        ADD)

                    def t_copy(ps_o=ps_o, h=h, qbp=qbp):
                        oraw = orawp.tile([P, 2 * QB], BF16, tag="oraw",
                                          bufs=1, name="oraw")
                        nc.scalar.activation(oraw[:], ps_o[:], AFT.Copy)
                        o_t[(h, qbp, "raw")] = oraw

                    def t_denom(acc=acc, h=h, qbp=qbp):
                        ps_d = psS()
                        for j in range(2):
                            nc.tensor.matmul(ps_d[:, j * QB:(j + 1) * QB],
                                             ones_t[:],
                                             acc[:, j * QB:(j + 1) * QB],
                                             start=True, stop=True)
                        o_t[(h, qbp, "den")] = ps_d

                    def t_rec(h=h, qbp=qbp):
                        # 1/d = exp(-ln d): both live in the same ACT table
                        # (no reloads), and keeping this OFF the DVE matters:
                        # a DVE reciprocal measured 6.5us and, DVE being
                        # strict FIFO, it stalled the acc adds -> e-buffer
                        # recycling -> exp -> PE for ~4us per query block.
                        lnd = stagep.tile([P, 2 * QB], F32, tag="s", bufs=2,
                                          name="lnd")
                        nc.scalar.activation(lnd[:],
                                             o_t.pop((h, qbp, "den"))[:],
                                             AFT.Ln)
                        rec = recp.tile([P, 2 * QB], BF16, tag="r", bufs=2,
                                        name="rec")
                        nc.scalar.activation(rec[:], lnd[:], AFT.Exp,
                                             scale=-1.0)
                        o_t[(h, qbp, "rec")] = rec

                    def t_norm(h=h, qbp=qbp):
                        o = opool.tile([P, 2 * QB], BF16, tag="o", bufs=16,
                                       name=f"o{h}_{qbp}")
                        with nc.allow_low_precision(reason="bf16 o tile"):
                            nc.vector.tensor_tensor(
                                o[:], o_t.pop((h, qbp, "raw"))[:],
                                o_t.pop((h, qbp, "rec"))[:], MUL)
                        o_t[(h, qbp)] = o

                    pending_tail = [t_copy, t_denom, t_rec, t_norm]
                # drain any unused filler units (shouldn't happen: 64 units
                # vs 64 slots, but keep the projections complete regardless)
                for u in filler:
                    u()
            for u in oq:
                u()
            for u in pending_tail:
                u()

            # ---- O-projection (seq tiles 8..15; 0..7 ran interleaved) ----
            for st in range(8, NKT):
                qbp, stq = st // 8, st % 8
                if st % 2 == 0:
                    pse = [psS(), psS()]
                else:
                    pse = [psO(), psQ()]
                for h in range(HPC):
                    lhs = o_t[(h, qbp)][:, stq * P:(stq + 1) * P]
                    for dbp in range(2):
                        for j in range(2):
                            c0 = dbp * GW + j * QB
                            nc.tensor.matmul(
                                pse[dbp][:, j * QB:(j + 1) * QB], lhs,
                                wo[h][:, c0:c0 + QB],
                                start=(h == 0), stop=(h == HPC - 1))
                ob = stagep.tile([P, D], BF16, tag="s", bufs=2, name="ob")
                for dbp in range(2):
                    nc.scalar.activation(ob[:, dbp * GW:(dbp + 1) * GW],
                                         pse[dbp][:], AFT.Copy)
                nc.gpsimd.dma_start(out[st * P:(st + 1) * P, :], ob[:])

    if split:
        split_waits(nc)
    return nc


_cache = {}


def prepare_in_maps(x, attn_mask, Wq, bq, Wk, bk, Wv, bv, Wo, bo):
    x = np.asarray(x, dtype=np.float32)
    attn_mask = np.asarray(attn_mask)
    Wq, bq = np.asarray(Wq, np.float32), np.asarray(bq, np.float32)
    Wk, bk = np.asarray(Wk, np.float32), np.asarray(bk, np.float32)
    Wv, bv_np = np.asarray(Wv, np.float32), np.asarray(bv, np.float32)
    Wo = np.asarray(Wo, np.float32)

    masked = not np.all(attn_mask == 1)
    scale = np.float32(1.0 / np.sqrt(DH))
    ones_np = np.ones((P, P), dtype=NPBF16)
    in_maps = []
    for c in range(N_CORES):
        b, g = c // 2, c % 2
        gs = slice(g * GW, (g + 1) * GW)
        xTb = np.ascontiguousarray(x[b].T)                       # (D, S)
        WqT = (Wq[gs, :] * scale).T                              # (D, GW)
        WkT = Wk[gs, :].T
        WvT = Wv[gs, :].T
        m = {
            "xT": xTb.astype(NPBF16),
            # WqP[hc][p, ki, d] = WqT[ki*128+p, hc*128+d]
            "WqP": np.ascontiguousarray(
                WqT.reshape(NKC, P, HPC, P).transpose(2, 1, 0, 3)
            ).astype(NPBF16),
            "WkP": np.ascontiguousarray(
                WkT.reshape(NKC, P, HPC, P).transpose(2, 1, 0, 3)
            ).astype(NPBF16),
            # WvP[w][p, j, d] = WvT[(2w+j)*128+p, d]
            "WvP": np.ascontiguousarray(
                WvT.reshape(8, 2, P, GW).transpose(0, 2, 1, 3)
            ).astype(NPBF16),
            "WoT": np.ascontiguousarray(Wo[:, gs].T).astype(NPBF16),
            "bqs": np.ascontiguousarray((bq[gs] * scale).reshape(HPC, P).T),
            "bks": np.ascontiguousarray(bk[gs].reshape(HPC, P).T),
            "bv": np.ascontiguousarray(
                np.broadcast_to(bv_np[gs][None, :], (P, GW))).astype(NPBF16),
            "ones": ones_np,
        }
        if masked:
            m["maskT"] = np.ascontiguousarray(
                attn_mask.reshape(S, S).T.astype(np.float32)).astype(NPBF16)
        in_maps.append(m)
    return in_maps, masked


def gather(parts, bo):
    parts = [np.asarray(p, dtype=np.float32) for p in parts]
    out = np.stack([parts[2 * b_] + parts[2 * b_ + 1] for b_ in range(B)])
    out += np.asarray(bo, np.float32)[None, None, :]
    return out.astype(np.float32)


def kernel(x, attn_mask, Wq, bq, Wk, bk, Wv, bv, Wo, bo):
    in_maps, masked = prepare_in_maps(x, attn_mask, Wq, bq, Wk, bk,
                                      Wv, bv, Wo, bo)
    key = ("nc", masked)
    if key not in _cache:
        _cache[key] = build(masked)
    nc = _cache[key]
    res = run_bass_kernel_spmd(nc, in_maps, list(range(N_CORES)))
    parts = [res.results[c]["out"] for c in range(N_CORES)]
    return gather(parts, bo)


# revision 24
# speedup vs baseline: 1.0031x; 1.0031x over previous
"""Multi-head attention (B=4, S=2048, D=2048, H=16) on 8 TRN2 NeuronCores.

Sharding: core c handles batch b=c//2 and head-group g=c%2 (8 of 16 heads).
Each core computes Q/K/V projections for its head group, attention, and a
partial O-projection (its heads' contribution to the full (S, D) output).
Host sums the two partials per batch and adds the output bias.

v2 design (vs the fp32r baseline):
 - ALL matmuls in bf16: same 1 cyc/row streaming as fp32r, but LDWEIGHTS
   gets Fast Weight Load (~4x cheaper than fp32r's ~200ns — the trace
   showed 265us of serialized fp32r LDWEIGHTS), and DMA/SBUF halve.
 - Everything stays resident in SBUF: x (64K/part), v (32K), q/k (rolling),
   o-partials (32K) — no DRAM round-trips for intermediates at all.
 - x is consumed *while it loads*: head 0's Q/K projections accumulate in
   PSUM as each x row-tile lands (x-load DMA ~25us == head-0 QK PE time).
 - Q/K projections of head h+1 are interleaved into the attention inner
   loop of head h as PE filler, so the scalar engine's exp latency
   (~1us per 128x1024 tile, the co-bottleneck of the baseline's attention
   phase) hides entirely under tensor work.
 - Softmax denominator: DVE accumulates exp tiles in bf16 (2x mode),
   one ones-matmul per query block partition-reduces it, DVE reciprocal
   (the recommended path) replaces the baseline's ln/exp chain, and the
   normalize multiply runs bf16 SBUF->SBUF at 2x off the critical path
   (PV output is staged raw via a scalar-engine copy to free PSUM fast).

PSUM (8 banks): tag S = [128,1024] x2 (scores / denom / V-proj / O-proj),
tag O = x1 (PV accumulator / warmup), tag Q = x1 (interleaved QK halves).
"""
import sys

for _p in ("/opt/trn_rl_repo", "/root/.axon_site/_ro/trn_rl_repo"):
    if _p not in sys.path:
        sys.path.insert(0, _p)

import numpy as np
import concourse.bass as bass
import concourse.tile as tile
import concourse.mybir as mybir
from concourse.bass_utils import run_bass_kernel_spmd

F32 = mybir.dt.float32
BF16 = mybir.dt.bfloat16
NPBF16 = mybir.dt.np(BF16)
AFT = mybir.ActivationFunctionType
ADD = mybir.AluOpType.add
MUL = mybir.AluOpType.mult

D = 2048       # model dim
S = 2048       # sequence length
B = 4          # batch
HPC = 8        # heads per core (16 heads / 2 tensor-parallel groups)
DH = 128       # head dim
GW = HPC * DH  # head-group width = 1024
N_CORES = 8
P = 128        # partitions
QB = 512       # matmul free-dim block (one PSUM bank of fp32)
NKT = S // P   # 16 key tiles
NKC = D // P   # 16 contraction chunks over model dim


def split_waits(nc, max_waits=1):
    """This walrus build rejects instructions with >1 semaphore wait
    (setupSyncWait: 'Too many sync wait commands').  Move extra waits onto
    NoOp instructions inserted just before, on the same engine stream —
    semantically identical (the engine stalls on the NoOps first)."""
    n = 0
    for fn in nc.m.functions:
        for blk in fn.blocks:
            out = []
            for inst in blk.instructions:
                si = inst.sync_info
                waits = list(si.on_wait) if si and si.on_wait else []
                eng = inst.engine
                if len(waits) > max_waits and eng != mybir.EngineType.Unassigned:
                    for w in waits[:-max_waits]:
                        nop = mybir.InstNoOp(name=nc.get_next_instruction_name())
                        nop.engine = eng
                        nop.sync_info = mybir.SyncInfo(on_wait=[w], on_update=[])
                        out.append(nop)
                        n += 1
                    inst.sync_info = mybir.SyncInfo(
                        on_wait=waits[-max_waits:],
                        on_update=list(si.on_update) if si.on_update else [],
                    )
                out.append(inst)
            blk.instructions = out
    return n


def build(masked=False, split=True):
    nc = bass.Bass("TRN2", target_bir_lowering=False, debug=False,
                   num_devices=N_CORES)

    xT = nc.dram_tensor("xT", [D, S], BF16, kind="ExternalInput").ap()
    WqP = nc.dram_tensor("WqP", [HPC, P, NKC, P], BF16, kind="ExternalInput").ap()
    WkP = nc.dram_tensor("WkP", [HPC, P, NKC, P], BF16, kind="ExternalInput").ap()
    WvP = nc.dram_tensor("WvP", [8, P, 2, GW], BF16, kind="ExternalInput").ap()
    WoT = nc.dram_tensor("WoT", [GW, D], BF16, kind="ExternalInput").ap()
    bqs = nc.dram_tensor("bqs", [P, HPC], F32, kind="ExternalInput").ap()
    bks = nc.dram_tensor("bks", [P, HPC], F32, kind="ExternalInput").ap()
    bv = nc.dram_tensor("bv", [P, GW], BF16, kind="ExternalInput").ap()
    ones = nc.dram_tensor("ones", [P, P], BF16, kind="ExternalInput").ap()
    if masked:
        maskT = nc.dram_tensor("maskT", [S, S], BF16, kind="ExternalInput").ap()
    out = nc.dram_tensor("out", [S, D], BF16, kind="ExternalOutput").ap()

    with tile.TileContext(nc) as tc:
        with (
            tc.tile_pool(name="xpool", bufs=16) as xpool,
            tc.tile_pool(name="vpool", bufs=1) as vpool,
            tc.tile_pool(name="qkp", bufs=2) as qkp,
            tc.tile_pool(name="wqkp", bufs=1) as wqkp,
            tc.tile_pool(name="wvp", bufs=8) as wvp,
            tc.tile_pool(name="opool", bufs=16) as opool,
            tc.tile_pool(name="orawp", bufs=1) as orawp,
            tc.tile_pool(name="expp", bufs=2) as expp,
            tc.tile_pool(name="accp", bufs=1) as accp,
            tc.tile_pool(name="recp", bufs=2) as recp,
            tc.tile_pool(name="stagep", bufs=2) as stagep,
            tc.tile_pool(name="misc", bufs=1) as misc,
            tc.tile_pool(name="psum", bufs=1, space="PSUM") as psum,
        ):
            def psS():
                return psum.tile([P, 2 * QB], F32, tag="S", bufs=2, name="psS")

            def psO():
                return psum.tile([P, 2 * QB], F32, tag="O", bufs=1, name="psO")

            def psQ2():
                # two rotating 1-bank tiles: the pair acts as the 1024-wide
                # "Q" region, but consecutive users double-buffer at bank
                # granularity (an o-proj unit's copy overlaps the next
                # unit's matmuls)
                return psum.tile([P, QB], F32, tag="Q2", bufs=2, name="psQ2")

            # ---- PE warm-up: no DMA dependency (memset'd junk operand), so
            # the PE ramps the HAM clock gate (~3.4us to 2.4 GHz) while the
            # first x/weight DMAs are still in flight.
            junk = misc.tile([P, P], BF16)
            nc.vector.memset(junk[:], 1.0)
            pw = psO()
            for _ in range(32):
                nc.tensor.matmul(pw[:, :P], junk[:], junk[:],
                                 start=True, stop=True)

            # ---- x load + head-0 Q/K projections (consume x as it lands) -
            # Each dma_start costs ~600ns on its issuing sequencer, so the
            # prelude burst (~30 issues) is spread across three queues:
            # weights on gpsimd, x row-tiles on scalar, Wv on vector.
            wq0 = wqkp.tile([P, NKC, P], BF16, tag="wq", name="wq0")
            nc.gpsimd.dma_start(wq0[:], WqP[0])
            wk0 = wqkp.tile([P, NKC, P], BF16, tag="wk", name="wk0")
            nc.gpsimd.dma_start(wk0[:], WkP[0])
            xk = []
            for ki in range(NKC):
                t = xpool.tile([P, S], BF16, tag="x", name=f"xk{ki}")
                nc.scalar.dma_start(t[:], xT[ki * P:(ki + 1) * P, :])
                xk.append(t)
            wv = []
            for w in range(8):
                t = wvp.tile([P, 2, GW], BF16, tag="wv", name=f"wv{w}")
                nc.sync.dma_start(t[:], WvP[w])
                wv.append(t)
            ones_t = misc.tile([P, P], BF16)
            nc.gpsimd.dma_start(ones_t[:], ones[:])
            bqs_t = misc.tile([P, HPC], F32)
            nc.gpsimd.dma_start(bqs_t[:], bqs[:])
            bks_t = misc.tile([P, HPC], F32)
            nc.gpsimd.dma_start(bks_t[:], bks[:])
            bv_t = misc.tile([P, GW], BF16)
            nc.gpsimd.dma_start(bv_t[:], bv[:])

            # accumulators: q halves on the two S banks-pairs, k half0 on O,
            # k half1 on the Q2 pair (4 quarter-banks)
            psq = [psS(), psS()]
            psk0 = psO()
            psk1 = [psQ2(), psQ2()]
            for ki in range(NKC):
                st_, sp_ = (ki == 0), (ki == NKC - 1)
                lhs = wq0[:, ki, :]
                for half in range(2):
                    for j in range(2):
                        c0 = half * GW + j * QB
                        nc.tensor.matmul(
                            psq[half][:, j * QB:(j + 1) * QB], lhs,
                            xk[ki][:, c0:c0 + QB], start=st_, stop=sp_)
                lhs = wk0[:, ki, :]
                for j in range(2):
                    nc.tensor.matmul(psk0[:, j * QB:(j + 1) * QB], lhs,
                                     xk[ki][:, j * QB:(j + 1) * QB],
                                     start=st_, stop=sp_)
                for j in range(2):
                    nc.tensor.matmul(psk1[j][:], lhs,
                                     xk[ki][:, GW + j * QB:GW + (j + 1) * QB],
                                     start=st_, stop=sp_)
            qh, kh = {}, {}
            qh[0] = qkp.tile([P, S], BF16, tag="q", name="qh0")
            kh[0] = qkp.tile([P, S], BF16, tag="k", name="kh0")
            for half in range(2):
                nc.scalar.activation(qh[0][:, half * GW:(half + 1) * GW],
                                     psq[half][:], AFT.Identity,
                                     bias=bqs_t[:, 0:1])
            nc.scalar.activation(kh[0][:, 0:GW], psk0[:], AFT.Identity,
                                 bias=bks_t[:, 0:1])
            for j in range(2):
                nc.scalar.activation(kh[0][:, GW + j * QB:GW + (j + 1) * QB],
                                     psk1[j][:], AFT.Identity,
                                     bias=bks_t[:, 0:1])

            # ---- V projection (v stays resident, natural (seq, dh)) ------
            v_all = vpool.tile([P, NKT, GW], BF16, tag="v", name="v_all")
            for st in range(NKT):
                ps = psS()
                for ki in range(NKC):
                    lhs = xk[ki][:, st * P:(st + 1) * P]
                    for j in range(2):
                        nc.tensor.matmul(
                            ps[:, j * QB:(j + 1) * QB], lhs,
                            wv[ki // 2][:, ki % 2, j * QB:(j + 1) * QB],
                            start=(ki == 0), stop=(ki == NKC - 1))
                with nc.allow_low_precision(reason="bf16 v tile"):
                    nc.vector.tensor_tensor(v_all[:, st, :], ps[:], bv_t[:],
                                            ADD)
            # Wo reuses the wv slots (same tag/size); WAR deps on the last
            # V-phase reads sequence the DMAs automatically.
            wo = []
            for h in range(HPC):
                t = wvp.tile([P, D], BF16, tag="wv", name=f"wo{h}")
                nc.gpsimd.dma_start(t[:], WoT[h * P:(h + 1) * P, :])
                wo.append(t)

            # ---- merged phase: attention(h) + QK-projections(h+1) --------
            o_t = {}
            pending_tail = []
            qk_ps = [None]  # current QK psum accumulator (tag Q, bufs=1)

            def qk_units_for(hn):
                """PE-filler units for head hn's q/k projections: 4
                subphases of 16 ki-steps each, 2 matmuls per step.  The
                weight DMAs issue eagerly (now); only the matmuls are
                deferred into the returned closures."""
                wqn = wqkp.tile([P, NKC, P], BF16, tag="wq", name=f"wq{hn}")
                nc.gpsimd.dma_start(wqn[:], WqP[hn])
                wkn = wqkp.tile([P, NKC, P], BF16, tag="wk", name=f"wk{hn}")
                nc.gpsimd.dma_start(wkn[:], WkP[hn])
                qh[hn] = qkp.tile([P, S], BF16, tag="q", name=f"qh{hn}")
                kh[hn] = qkp.tile([P, S], BF16, tag="k", name=f"kh{hn}")
                units = []
                for w, bias_t, dst in ((wqn, bqs_t, qh[hn]),
                                       (wkn, bks_t, kh[hn])):
                    for half in range(2):
                        for ki in range(NKC):
                            def unit(w=w, bias_t=bias_t, dst=dst, half=half,
                                     ki=ki, hn=hn):
                                if ki == 0:
                                    # allocated at emission time so the
                                    # Q2 slots' WAR deps land in order
                                    qk_ps[0] = [psQ2(), psQ2()]
                                for j in range(2):
                                    nc.tensor.matmul(
                                        qk_ps[0][j][:], w[:, ki, :],
                                        xk[ki][:, half * GW + j * QB:
                                               half * GW + (j + 1) * QB],
                                        start=(ki == 0), stop=(ki == NKC - 1))
                                if ki == NKC - 1:
                                    for j in range(2):
                                        c0 = half * GW + j * QB
                                        nc.scalar.activation(
                                            dst[:, c0:c0 + QB],
                                            qk_ps[0][j][:], AFT.Identity,
                                            bias=bias_t[:, hn:hn + 1])
                            units.append(unit)
                return units

            def oproj_half_unit(st, dbp):
                """One half (1024 out cols) of seq-tile st's O-projection,
                as two quarter-bank accumulation groups on the Q2 rotation:
                the DVE copy of one bank overlaps the matmuls into the
                other, so back-to-back units never stall the PE.  Used both
                interleaved into the last attention block (whose kt loop is
                otherwise ACT-gated) and for the trailing seq tiles."""
                def unit():
                    qb0, stq = st // 8, st % 8
                    ob = stagep.tile([P, GW], BF16, tag="s", bufs=2,
                                     name="obh")
                    for j in range(2):
                        ps = psQ2()
                        c0 = dbp * GW + j * QB
                        for hh in range(HPC):
                            lhs = o_t[(hh, qb0)][:, stq * P:(stq + 1) * P]
                            nc.tensor.matmul(ps[:], lhs,
                                             wo[hh][:, c0:c0 + QB],
                                             start=(hh == 0),
                                             stop=(hh == HPC - 1))
                        with nc.allow_low_precision(reason="bf16 out tile"):
                            nc.vector.tensor_copy(ob[:, j * QB:(j + 1) * QB],
                                                  ps[:])
                    nc.gpsimd.dma_start(
                        out[st * P:(st + 1) * P, dbp * GW:(dbp + 1) * GW],
                        ob[:])
                return unit

            oq = [oproj_half_unit(st, dbp)
                  for st in range(8) for dbp in range(2)]

            for h in range(HPC):
                filler = iter(qk_units_for(h + 1) if h < HPC - 1 else [])
                for qbp in range(2):
                    acc = accp.tile([P, 2 * QB], BF16, tag="a", bufs=1,
                                    name="acc")
                    ps_o = psO()
                    for kt in range(NKT):
                        st_, sp_ = (kt == 0), (kt == NKT - 1)
                        ps_s = psS()
                        ks = kh[h][:, kt * P:(kt + 1) * P]
                        for j in range(2):
                            c0 = qbp * GW + j * QB
                            nc.tensor.matmul(ps_s[:, j * QB:(j + 1) * QB],
                                             ks, qh[h][:, c0:c0 + QB],
                                             start=True, stop=True)
                        e = expp.tile([P, 2 * QB], BF16, tag="e", bufs=2,
                                      name="e")
                        nc.scalar.activation(e[:], ps_s[:], AFT.Exp)
                        if masked:
                            m = orawp.tile([P, 2 * QB], BF16, tag="oraw",
                                           bufs=2, name="mtile")
                            nc.gpsimd.dma_start(
                                m[:], maskT[kt * P:(kt + 1) * P,
                                            qbp * GW:(qbp + 1) * GW])
                            with nc.allow_low_precision(reason="mask mul"):
                                nc.vector.tensor_tensor(e[:], e[:], m[:], MUL)
                        # deferred tail of the previous query block, spread
                        # over this block's first kts.  t_copy AND t_denom
                        # must both land at kt==0: they read the previous
                        # block's ps_o/acc, whose single-buffered slots this
                        # block's PV(kt0)/acc-copy (emitted below) reuse.
                        if pending_tail:
                            if kt == 0:
                                pending_tail.pop(0)()
                                pending_tail.pop(0)()
                            elif kt <= 2:
                                pending_tail.pop(0)()
                        # PE filler while ACT runs exp(kt): QK ki-steps for
                        # head h+1, or (last block) O-projection halves
                        if h < HPC - 1:
                            for _ in range(2):
                                u = next(filler, None)
                                if u is not None:
                                    u()
                        elif qbp == 1 and kt >= 4 and oq:
                            oq.pop(0)()
                        vs = v_all[:, kt, h * DH:(h + 1) * DH]
                        for j in range(2):
                            nc.tensor.matmul(ps_o[:, j * QB:(j + 1) * QB],
                                             vs, e[:, j * QB:(j + 1) * QB],
                                             start=st_, stop=sp_)
                        with nc.allow_low_precision(reason="bf16 denom acc"):
                            if kt == 0:
                                nc.vector.tensor_copy(acc[:], e[:])
                            else:
                                nc.vector.tensor_tensor(acc[:], acc[:], e[:],
                                                        ADD)

                    def t_copy(ps_o=ps_o, h=h, qbp=qbp):
                        oraw = orawp.tile([P, 2 * QB], BF16, tag="oraw",
                                          bufs=1, name="oraw")
                        nc.scalar.activation(oraw[:], ps_o[:], AFT.Copy)
                        o_t[(h, qbp, "raw")] = oraw

                    def t_denom(acc=acc, h=h, qbp=qbp):
                        ps_d = psS()
                        for j in range(2):
                            nc.tensor.matmul(ps_d[:, j * QB:(j + 1) * QB],
                                             ones_t[:],
                                             acc[:, j * QB:(j + 1) * QB],
                                             start=True, stop=True)
                        o_t[(h, qbp, "den")] = ps_d

                    def t_rec(h=h, qbp=qbp):
                        # 1/d = exp(-ln d): both live in the same ACT table
                        # (no reloads), and keeping this OFF the DVE matters:
                        # a DVE reciprocal measured 6.5us and, DVE being
                        # strict FIFO, it stalled the acc adds -> e-buffer
                        # recycling -> exp -> PE for ~4us per query block.
                        lnd = stagep.tile([P, 2 * QB], F32, tag="s", bufs=2,
                                          name="lnd")
                        nc.scalar.activation(lnd[:],
                                             o_t.pop((h, qbp, "den"))[:],
                                             AFT.Ln)
                        rec = recp.tile([P, 2 * QB], BF16, tag="r", bufs=2,
                                        name="rec")
                        nc.scalar.activation(rec[:], lnd[:], AFT.Exp,
                                             scale=-1.0)
                        o_t[(h, qbp, "rec")] = rec

                    def t_norm(h=h, qbp=qbp):
                        o = opool.tile([P, 2 * QB], BF16, tag="o", bufs=16,
                                       name=f"o{h}_{qbp}")
                        with nc.allow_low_precision(reason="bf16 o tile"):
                            nc.vector.tensor_tensor(
                                o[:], o_t.pop((h, qbp, "raw"))[:],
                                o_t.pop((h, qbp, "rec"))[:], MUL)
                        o_t[(h, qbp)] = o

                    pending_tail = [t_copy, t_denom, t_rec, t_norm]
                # drain any unused filler units (shouldn't happen: 64 units
                # vs 64 slots, but keep the projections complete regardless)
                for u in filler:
                    u()
            for u in oq:
                u()
            for u in pending_tail:
                u()

            # ---- O-projection (seq tiles 8..15; 0..7 ran interleaved) ----
            for st in range(8, NKT):
                for dbp in range(2):
                    oproj_half_unit(st, dbp)()

    if split:
        split_waits(nc)
    return nc


_cache = {}


def prepare_in_maps(x, attn_mask, Wq, bq, Wk, bk, Wv, bv, Wo, bo):
    x = np.asarray(x, dtype=np.float32)
    attn_mask = np.asarray(attn_mask)
    Wq, bq = np.asarray(Wq, np.float32), np.asarray(bq, np.float32)
    Wk, bk = np.asarray(Wk, np.float32), np.asarray(bk, np.float32)
    Wv, bv_np = np.asarray(Wv, np.float32), np.asarray(bv, np.float32)
    Wo = np.asarray(Wo, np.float32)

    masked = not np.all(attn_mask == 1)
    scale = np.float32(1.0 / np.sqrt(DH))
    ones_np = np.ones((P, P), dtype=NPBF16)
    in_maps = []
    for c in range(N_CORES):
        b, g = c // 2, c % 2
        gs = slice(g * GW, (g + 1) * GW)
        xTb = np.ascontiguousarray(x[b].T)                       # (D, S)
        WqT = (Wq[gs, :] * scale).T                              # (D, GW)
        WkT = Wk[gs, :].T
        WvT = Wv[gs, :].T
        m = {
            "xT": xTb.astype(NPBF16),
            # WqP[hc][p, ki, d] = WqT[ki*128+p, hc*128+d]
            "WqP": np.ascontiguousarray(
                WqT.reshape(NKC, P, HPC, P).transpose(2, 1, 0, 3)
            ).astype(NPBF16),
            "WkP": np.ascontiguousarray(
                WkT.reshape(NKC, P, HPC, P).transpose(2, 1, 0, 3)
            ).astype(NPBF16),
            # WvP[w][p, j, d] = WvT[(2w+j)*128+p, d]
            "WvP": np.ascontiguousarray(
                WvT.reshape(8, 2, P, GW).transpose(0, 2, 1, 3)
            ).astype(NPBF16),
            "WoT": np.ascontiguousarray(Wo[:, gs].T).astype(NPBF16),
            "bqs": np.ascontiguousarray((bq[gs] * scale).reshape(HPC, P).T),
            "bks": np.ascontiguousarray(bk[gs].reshape(HPC, P).T),
            "bv": np.ascontiguousarray(
                np.broadcast_to(bv_np[gs][None, :], (P, GW))).astype(NPBF16),
            "ones": ones_np,
        }
        if masked:
            m["maskT"] = np.ascontiguousarray(
                attn_mask.reshape(S, S).T.astype(np.float32)).astype(NPBF16)
        in_maps.append(m)
    return in_maps, masked


def gather(parts, bo):
    parts = [np.asarray(p, dtype=np.float32) for p in parts]
    out = np.stack([parts[2 * b_] + parts[2 * b_ + 1] for b_ in range(B)])
    out += np.asarray(bo, np.float32)[None, None, :]
    return out.astype(np.float32)


def kernel(x, attn_mask, Wq, bq, Wk, bk, Wv, bv, Wo, bo):
    in_maps, masked = prepare_in_maps(x, attn_mask, Wq, bq, Wk, bk,
                                      Wv, bv, Wo, bo)
    key = ("nc", masked)
    if key not in _cache:
        _cache[key] = build(masked)
    nc = _cache[key]
    res = run_bass_kernel_spmd(nc, in_maps, list(range(N_CORES)))
    parts = [res.results[c]["out"] for c in range(N_CORES)]
    return gather(parts, bo)
